# revision 61
# baseline (speedup 1.0000x reference)
"""Trainium2 Bass kernel for nn_DilatedSparseRnnStack.

Data-parallel over batch: 8 cores x 128 series each. Everything SBUF-resident:
weights (bf16), packed transposed input x, dilated-LSTM state circular buffers.
Per timestep: PE matmuls (bf16 in, fp32 PSUM) -> ScalarE sigmoid/tanh (one op
per gate, fused across layer pairs, ordered by when the cell chain consumes
each gate) -> VectorE cell-state chain (bf16, gate-free subtract on GpSimd) ->
transposes back to feature-major: h via PE transpose-mode into the just-freed
gates PSUM slot at the start of the NEXT superstep (short recurrence latency),
layer outputs via DMA-xbar transpose (a full superstep of slack). Layers are
software-pipelined (layer L processes timestep s-L at superstep s) so all four
layers' work overlaps across engines. Final projection (Wout) runs as one
batched matmul phase at the end.
"""

import sys

sys.path.insert(0, "/opt/trn_rl_repo")

import numpy as np
import ml_dtypes

import concourse.bacc as bacc
import concourse.tile as tile
import concourse.mybir as mybir
from concourse.bass_utils import run_bass_kernel_spmd

BF16 = ml_dtypes.bfloat16

# Model config (hardcoded per problem spec)
DILS = [1, 3, 6, 12]
IN, SS, HS = 64, 256, 128
OS = SS - HS          # 128
OUT = 8
B, T = 1024, 256
NCORES = 8
BL = B // NCORES      # 128 batch rows per core
G4 = 4 * SS           # 1024 gate width

F32 = mybir.dt.float32
BF = mybir.dt.bfloat16
AF = mybir.ActivationFunctionType
Alu = mybir.AluOpType

# Per-layer input-piece column layout inside W (fan-in axis)
#   L0: x[0:64]   h[64:192]  d[192:320]
#   L1: o[0:128]  h[128:256] d[256:384]
#   L2: o[0:128]  x[128:192] h[192:320] d[320:448]
#   L3: o[0:128]  h[128:256] d[256:384]
PIECES = [
    {"x": (0, 64), "h": (64, 192), "d": (192, 320)},
    {"o": (0, 128), "h": (128, 256), "d": (256, 384)},
    {"o": (0, 128), "x": (128, 192), "h": (192, 320), "d": (320, 448)},
    {"o": (0, 128), "h": (128, 256), "d": (256, 384)},
]


def _perm_rows(W):
    """Reorder gate blocks [g0,g1,g2,g3] -> [g1(tanh), g0(sig+1), g2(sig), g3(sig)]."""
    return np.concatenate([W[SS:2 * SS], W[0:SS], W[2 * SS:3 * SS], W[3 * SS:4 * SS]], axis=0)


def prep_host_inputs(inputs, Tn=T):
    """Build the device input arrays (weights shared across cores; x per core)."""
    shared = {}
    for li in range(4):
        W = _perm_rows(np.asarray(inputs[f"W{li}"], np.float32))
        p = PIECES[li]
        if "x" in p:
            a, b = p["x"]
            wxT = np.ascontiguousarray(W[:, a:b].T).astype(BF16)
            # duplicated in rows 64:128 so odd-t lhsT (base_partition 64) can
            # read the rhs at the same base partition (matmul requirement)
            shared[f"w{li}x"] = np.concatenate([wxT, wxT], axis=0)
        if "o" in p:
            a, b = p["o"]
            shared[f"w{li}o"] = np.ascontiguousarray(W[:, a:b].T).astype(BF16)
        ha, hb = p["h"]
        da, db = p["d"]
        Wh, Wd = W[:, ha:hb], W[:, da:db]
        shared[f"w{li}hd"] = np.ascontiguousarray((Wh + Wd).T).astype(BF16)
        if li > 0:
            shared[f"w{li}h"] = np.ascontiguousarray(Wh.T).astype(BF16)
            shared[f"w{li}d"] = np.ascontiguousarray(Wd.T).astype(BF16)
    shared["wout"] = np.ascontiguousarray(np.asarray(inputs["Wout"], np.float32).T).astype(BF16)
    shared["ident"] = np.eye(128, dtype=BF16)

    for li in range(4):
        bvec = np.asarray(inputs[f"b{li}"], np.float32)
        if np.any(bvec != 0.0):
            bb = _perm_rows(bvec.reshape(-1, 1)).reshape(-1)
            shared[f"bias{li}"] = np.ascontiguousarray(
                np.broadcast_to(bb[None, :], (BL, G4))
            ).astype(np.float32)

    x = np.asarray(inputs["x"], np.float32)
    per_core = []
    for c in range(NCORES):
        xs = x[:Tn, c * BL:(c + 1) * BL, :]                     # [T, BL, 64]
        xpk = (
            xs.reshape(Tn // 2, 2, BL, IN)
            .transpose(1, 3, 0, 2)                              # [2, 64, T/2, BL]
            .reshape(128, (Tn // 2) * BL)
        )
        per_core.append({"xpk": np.ascontiguousarray(xpk).astype(BF16)})
    return shared, per_core


def build_program(Tn=T, bias_layers=()):
    """Trace the Bass/Tile program for sequence length Tn. Returns nc."""
    nc = bacc.Bacc("TRN2", target_bir_lowering=False, debug=False)

    # ---- DRAM I/O ----
    dws = {}
    for li in range(4):
        p = PIECES[li]
        if "x" in p:
            dws[f"w{li}x"] = nc.dram_tensor(f"w{li}x", [128, G4], BF, kind="ExternalInput")
        if "o" in p:
            dws[f"w{li}o"] = nc.dram_tensor(f"w{li}o", [128, G4], BF, kind="ExternalInput")
        dws[f"w{li}hd"] = nc.dram_tensor(f"w{li}hd", [128, G4], BF, kind="ExternalInput")
        if li > 0:
            dws[f"w{li}h"] = nc.dram_tensor(f"w{li}h", [128, G4], BF, kind="ExternalInput")
            dws[f"w{li}d"] = nc.dram_tensor(f"w{li}d", [128, G4], BF, kind="ExternalInput")
    dws["wout"] = nc.dram_tensor("wout", [OS, OUT], BF, kind="ExternalInput")
    dws["ident"] = nc.dram_tensor("ident", [128, 128], BF, kind="ExternalInput")
    for li in bias_layers:
        dws[f"bias{li}"] = nc.dram_tensor(f"bias{li}", [BL, G4], F32, kind="ExternalInput")
    d_xpk = nc.dram_tensor("xpk", [128, (Tn // 2) * BL], BF, kind="ExternalInput")
    d_y = nc.dram_tensor("y", [OUT, Tn * BL], F32, kind="ExternalOutput")

    with tile.TileContext(nc) as tc:
        from contextlib import ExitStack

        with ExitStack() as ctx:
            wpool = ctx.enter_context(tc.tile_pool(name="wpool", bufs=1))
            xpool = ctx.enter_context(tc.tile_pool(name="xpool", bufs=1))
            spool = ctx.enter_context(tc.tile_pool(name="spool", bufs=1))
            gspool = ctx.enter_context(tc.tile_pool(name="gspool", bufs=3))
            whpool = ctx.enter_context(tc.tile_pool(name="whpool", bufs=3))
            tpool = ctx.enter_context(tc.tile_pool(name="tpool", bufs=3))
            yspool = ctx.enter_context(tc.tile_pool(name="yspool", bufs=2))

            # ---- load weights into SBUF ----
            wt = {}
            for name, dt_ in dws.items():
                if name.startswith("bias"):
                    w_tile = wpool.tile([BL, G4], F32, tag=name, name=name + "_s")
                else:
                    w_tile = wpool.tile(list(dt_.shape), BF, tag=name, name=name + "_s")
                nc.sync.dma_start(out=w_tile[:], in_=dt_.ap())
                wt[name] = w_tile

            # ---- load packed x (chunked DMAs so early steps start sooner) ----
            xt = xpool.tile([128, (Tn // 2) * BL], BF, tag="xt")
            ncols = (Tn // 2) * BL
            nchunk = max(1, min(16, ncols // 1024))
            cw = ncols // nchunk
            for i in range(nchunk):
                a, b = i * cw, (i + 1) * cw if i < nchunk - 1 else ncols
                nc.sync.dma_start(out=xt[:, a:b], in_=d_xpk.ap()[:, a:b])

            # ---- persistent state slots ----
            # TO[l][k]: [128, 2, BL] bf16 ring — [:,0,:]=out.T, [:,1,:]=h.T,
            # both written by ONE combined dma transpose of `whole`
            Cs = []  # Cs[l][k]: [BL, SS] bf16 cell state ring
            TO = []
            for li, d in enumerate(DILS):
                Cs.append([spool.tile([BL, SS], BF, tag=f"C{li}_{k}", name=f"C{li}_{k}")
                           for k in range(d)])
                TO.append([spool.tile([128, 2, BL], BF, tag=f"T{li}_{k}",
                                      name=f"T{li}_{k}") for k in range(d)])
            # L3 out.T accumulates here for the batched end-phase projection
            o3 = spool.tile([OS, Tn * BL], BF, tag="o3", name="o3")

            out_tiles = [None, None, None]  # out.T APs of layers 0..2 from prev superstep

            whole_prev = [None, None, None, None]  # whole tiles from superstep s-1
            with tc.tile_pool(name="gppool", bufs=1, space="PSUM") as gppool:
                for s in range(Tn + 4):
                    new_out = [None, None, None]
                    new_whole = [None, None, None, None]
                    usub = [None, None, None, None]
                    for pi, pair in enumerate(((0, 1), (2, 3))):
                        valid = [(w, l, s - l) for w, l in enumerate(pair)
                                 if 0 <= s - l < Tn]
                        pvalid = [(w, l, s - 1 - l) for w, l in enumerate(pair)
                                  if 0 <= s - 1 - l <= Tn - 2]
                        # --- h.T of the previous superstep via PE transpose
                        # into this pair's just-freed PSUM slot, evacuated on
                        # DVE straight into the h ring (short-latency path) ---
                        if pvalid:
                            hps = gppool.tile([BL, 2 * G4], BF, tag=f"gp{pi}",
                                              name=f"hps{pi}_{s}")
                            hpsv = hps.rearrange("p (l f) -> p l f", l=2)
                            for w, li, tp in pvalid:
                                nc.tensor.transpose(
                                    hpsv[:, w, 0:BL],
                                    whole_prev[li][:, OS:SS], wt["ident"][:])
                            for w, li, tp in pvalid:
                                nc.vector.tensor_copy(
                                    TO[li][tp % DILS[li]][:, 1, :], hpsv[:, w, 0:BL])
                        if not valid:
                            continue
                        gp = gppool.tile([BL, 2 * G4], F32, tag=f"gp{pi}",
                                         name=f"gp{pi}_{s}")
                        gs = gspool.tile([BL, 2 * G4], BF, tag=f"gs{pi}",
                                         name=f"gs{pi}_{s}")

                        # gate-free subtracts early on Pool: operands are
                        # last step's states, ready before the matmuls
                        for w, li, t in valid:
                            d = DILS[li]
                            if t >= d and li != 0:
                                ta = tpool.tile([BL, SS], BF, tag=f"ta{li}",
                                                name=f"ta{li}_{s}")
                                nc.gpsimd.tensor_sub(
                                    ta[:], Cs[li][(t - 1) % d][:], Cs[li][t % d][:])
                                usub[li] = ta

                        # --- matmuls: gates[t] for each valid layer of the pair ---
                        for w, li, t in valid:
                            d = DILS[li]
                            cur, prv = t % d, (t - 1) % d
                            pieces = []
                            if li in (1, 2, 3):
                                pieces.append((out_tiles[li - 1], wt[f"w{li}o"]))
                            if li in (0, 2):
                                r0 = (t % 2) * 64
                                c0 = (t // 2) * BL
                                pieces.append((xt[r0:r0 + 64, c0:c0 + BL],
                                               wt[f"w{li}x"][r0:r0 + 64, :]))
                            if t >= 1:
                                if t < d or li == 0:
                                    pieces.append((TO[li][prv][:, 1, :], wt[f"w{li}hd"]))
                                else:
                                    pieces.append((TO[li][prv][:, 1, :], wt[f"w{li}h"]))
                                    pieces.append((TO[li][cur][:, 1, :], wt[f"w{li}d"]))
                            base = w * G4
                            for i, (lhsT, rhs) in enumerate(pieces):
                                first, last = i == 0, i == len(pieces) - 1
                                for n in range(2):
                                    nc.tensor.matmul(
                                        out=gp[:, base + n * 512: base + (n + 1) * 512],
                                        lhsT=lhsT,
                                        rhs=rhs[:, n * 512:(n + 1) * 512],
                                        start=first,
                                        stop=last,
                                    )
                            if li in bias_layers:
                                nc.vector.tensor_add(
                                    gp[:, base:base + G4],
                                    gp[:, base:base + G4],
                                    wt[f"bias{li}"][:],
                                )

                        # --- activations (fused across the pair when both valid) ---
                        gpv = gp.rearrange("p (l f) -> p l f", l=2)
                        gsv = gs.rearrange("p (l f) -> p l f", l=2)
                        if len(valid) == 2:
                            isel = slice(0, 2)
                        else:
                            isel = slice(valid[0][0], valid[0][0] + 1)
                        # one op per gate (pair-fused), ordered by when the
                        # DVE chain consumes each gate: alpha, cand, forget, og
                        nc.scalar.activation(
                            out=gsv[:, isel, 2 * SS:3 * SS],
                            in_=gpv[:, isel, 2 * SS:3 * SS], func=AF.Sigmoid)
                        nc.scalar.activation(
                            out=gsv[:, isel, 0:SS], in_=gpv[:, isel, 0:SS], func=AF.Tanh)
                        nc.scalar.activation(
                            out=gsv[:, isel, SS:2 * SS], in_=gpv[:, isel, SS:2 * SS],
                            func=AF.Sigmoid, bias=1.0)
                        nc.scalar.activation(
                            out=gsv[:, isel, 3 * SS:4 * SS],
                            in_=gpv[:, isel, 3 * SS:4 * SS], func=AF.Sigmoid)

                        # --- cell-state chain per valid layer ---
                        for w, li, t in valid:
                            d = DILS[li]
                            cur, prv = t % d, (t - 1) % d
                            cand = gsv[:, w, 0:SS]
                            fg = gsv[:, w, SS:2 * SS]
                            al = gsv[:, w, 2 * SS:3 * SS]
                            og = gsv[:, w, 3 * SS:4 * SS]
                            whole = whpool.tile([BL, SS], BF, tag=f"wh{li}",
                                                name=f"wh{li}_{s}")
                            if t == 0:
                                nc.vector.tensor_copy(Cs[li][cur][:], cand)
                                nc.vector.tensor_mul(whole[:], og, cand)
                            else:
                                tb = tpool.tile([BL, SS], BF, tag=f"tb{li}",
                                                name=f"tb{li}_{s}")
                                if t >= d and li != 0:
                                    # wC = dC + a*(prevC - dC); u precomputed
                                    ta = usub[li]
                                    nc.vector.tensor_mul(tb[:], al, ta[:])
                                    nc.vector.tensor_add(ta[:], tb[:], Cs[li][cur][:])
                                    wC = ta[:]
                                else:
                                    wC = Cs[li][prv][:]
                                # newC = cand + f*(wC - cand)
                                nc.vector.tensor_sub(tb[:], wC, cand)
                                nc.vector.tensor_mul(tb[:], fg, tb[:])
                                nc.vector.tensor_add(Cs[li][cur][:], tb[:], cand)
                                # single full-width output multiply: the PE
                                # transpose runs at the next superstep's start,
                                # so the h-half no longer needs to finish early
                                nc.vector.tensor_mul(whole[:], og, Cs[li][cur][:])

                            # --- out.T via DMA transpose (a full superstep of
                            # slack); h.T handled next superstep on the PE ---
                            new_whole[li] = whole
                            if li < 3:
                                nc.sync.dma_start_transpose(
                                    TO[li][cur][:, 0, :], whole[:, 0:OS])
                                new_out[li] = TO[li][cur][:, 0, :]
                            else:
                                nc.sync.dma_start_transpose(
                                    o3[:, t * BL:(t + 1) * BL], whole[:, 0:OS])
                    out_tiles = new_out
                    whole_prev = new_whole

            # ---- end phase: y.T = Wout @ out3.T, chunked ----
            with tc.tile_pool(name="ypsum", bufs=2, space="PSUM") as ypsum:
                CH = 512
                for c0 in range(0, Tn * BL, CH):
                    yp = ypsum.tile([OUT, CH], F32, tag="yp", name=f"yp_{c0}")
                    nc.tensor.matmul(out=yp[:], lhsT=wt["wout"][:],
                                     rhs=o3[:, c0:c0 + CH], start=True, stop=True)
                    ys = yspool.tile([OUT, CH], F32, tag="ystage", name=f"ys_{c0}")
                    nc.vector.tensor_copy(ys[:], yp[:])
                    nc.sync.dma_start(out=d_y.ap()[:, c0:c0 + CH], in_=ys[:])

    nc.compile()
    return nc


def kernel(**inputs):
    Tn = T
    bias_layers = tuple(
        li for li in range(4) if np.any(np.asarray(inputs[f"b{li}"], np.float32) != 0.0)
    )
    shared, per_core = prep_host_inputs(inputs, Tn)
    nc = build_program(Tn, bias_layers)
    in_maps = [dict(shared, **pc) for pc in per_core]
    res = run_bass_kernel_spmd(nc, in_maps, core_ids=list(range(NCORES)))
    outs = []
    for c in range(NCORES):
        yT = res.results[c]["y"]                     # [8, T*BL]
        outs.append(yT.reshape(OUT, Tn, BL).transpose(1, 2, 0))  # [T, BL, 8]
    y = np.concatenate(outs, axis=1).astype(np.float32)          # [T, B, 8]
    bout = np.asarray(inputs["bout"], np.float32)
    if np.any(bout != 0.0):
        y = y + bout[None, None, :]
    return y
